# revision 24
# baseline (speedup 1.0000x reference)
"""DSNT double loss kernel for Trainium2 (Bass/Tile), data-parallel over batch.

Problem: input/target [B=32, C=8, H=256, W=256] f32.
  probs = softmax(input.reshape(B,C,H*W)); pred_x = sum(probs * (w+1)/W);
  pred_y = sum(probs * (h+1)/H); am = argmax(target); true coords from am;
  loss = sum(sqrt((tx-px)^2+(ty-py)^2)) / B, shape (1,).

Sharding: batch dim across 8 cores (4 batches/core = 32 heatmaps/core).
Per-core layout: the 32 heatmaps (65536 px each) are viewed as a [128, 16384]
f32 block: partition p holds quarter q = p%4 of heatmap r = p//4 (contiguous
reshape).

Per-core dataflow (one pass over the data, DMA-bound):
  - input chunks: ACT computes exp with per-heatmap-row accumulators (scol);
    DVE does one fused multiply+accumulate pass against the x-weights (Sx).
  - target chunks: DVE computes per-heatmap-row maxima (Rall) in one
    tensor_reduce per chunk. The argmax is then recovered cheaply: V =
    max(Rall); k* = find_index8(V, Rall) gives the row; one indirect DMA
    gathers that 256-px row from DRAM; j* = find_index8(V, row) gives the
    column. Exact because the target has no duplicated per-line maxima.
  - cross-quarter (4 partitions per heatmap) merge via stream_shuffle
    rotations inside partition groups of 4, then one PE matmul against a
    0/1 row-selector folds [128] partitions to [32] heatmaps; a second
    1-column matmul sums the 32 EDs to the scalar partial.
"""

import numpy as np

_B, _C, _H, _W = 32, 8, 256, 256
_NCORES = 8
_P = 128
_ROWS = (_B // _NCORES) * _C          # 32 heatmaps per core
_ROWLEN = _H * _W                     # 65536
_FREE = _ROWS * _ROWLEN // _P         # 16384 elements per partition
_CHUNK = 2048
_NCHUNK = _FREE // _CHUNK             # 8
_HROWS = _FREE // _W                  # 64 heatmap-rows per partition
# ramp-up / drain-friendly chunk schedule (sums to _FREE, all multiples of W)
_SIZES = [512, 1536, 2048, 2048, 2048, 2048, 2048, 2048, 1024, 512, 512]
assert sum(_SIZES) == _FREE and all(sz % _W == 0 for sz in _SIZES)

_compiled = None
_tile_patched = False


def _patch_tile_single_wait():
    """The walrus build in this image encodes at most ONE semaphore wait per
    instruction ("Too many sync wait commands" / "ISA wrong length" errors in
    codegen otherwise). Split any multi-wait instruction into single-wait
    same-engine NOPs inserted immediately before it, and do the same for the
    kernel-tail drain."""
    global _tile_patched
    if _tile_patched:
        return
    _tile_patched = True

    import bass_rust
    from concourse import tile, mybir
    from concourse.vector_clock import ScopedClock

    ctr = [0]

    def split_waits(tc, inst):
        si = inst.sync_info
        if si is None or not si.on_wait or len(si.on_wait) <= 1:
            return
        wl = list(si.on_wait)
        si.on_wait = wl[-1:]
        for w in wl[:-1]:
            ctr[0] += 1
            nop = mybir.InstNoOp(name=f"wsplit-{ctr[0]}", ins=[], outs=[])
            nop.engine = inst.engine
            nop.sync_info = bass_rust.SyncInfo(on_wait=[w], on_update=[])
            tc._add_instruction(nop)

    orig_commit = tile.TileContext._commit_instruction

    def commit(self, inst, lazy_reg_writes=True):
        si = inst.sync_info
        if (si is not None and si.on_wait and len(si.on_wait) > 1
                and inst.engine != mybir.EngineType.Unassigned):
            split_waits(self, inst)
        return orig_commit(self, inst, lazy_reg_writes)

    tile.TileContext._commit_instruction = commit

    def drain_and_barrier(self, tick_clock, wait_clock):
        nc = self.nc
        drain_inst = nc.sync.drain()
        wait_clock.add_sem_waits(
            drain_inst.ins, ScopedClock({None: tick_clock.global_clock}))
        si = drain_inst.ins.sync_info
        wl = list(si.on_wait or []) if si else []
        if len(wl) > 1:
            si.on_wait = wl[:1]
            for w in wl[1:]:
                ctr[0] += 1
                nop = mybir.InstNoOp(name=f"dsplit-{ctr[0]}", ins=[], outs=[])
                nop.engine = mybir.EngineType.SP
                nop.sync_info = bass_rust.SyncInfo(on_wait=[w], on_update=[])
                self._add_instruction(nop)
        nc.all_engine_barrier()
        assert self.sems is not None
        popped = nc._tile_sem_poison_stack.pop()
        assert popped is self._sem_poison
        nc.clear_and_free_semaphores(list(self.sems.allocated().values()))
        nc.all_engine_barrier()

    tile.TileContext._drain_and_barrier = drain_and_barrier


# stream_shuffle masks: rotate by 1 / 2 within each partition group of 4
_ROT1 = [(i & ~3) | ((i + 1) & 3) for i in range(32)]
_ROT2 = [(i & ~3) | ((i + 2) & 3) for i in range(32)]


def _build_program():
    from concourse import bass, tile, mybir

    _patch_tile_single_wait()

    f32 = mybir.dt.float32
    u32 = mybir.dt.uint32
    Alu = mybir.AluOpType
    Act = mybir.ActivationFunctionType
    Ax = mybir.AxisListType

    nc = bass.Bass()
    inp = nc.declare_dram_parameter("inp", [_P, _FREE], f32, isOutput=False)
    tgt = nc.declare_dram_parameter("tgt", [_P, _FREE], f32, isOutput=False)
    xf = nc.declare_dram_parameter("xf", [_P, _CHUNK], f32, isOutput=False)
    yw = nc.declare_dram_parameter("yw", [_P, _HROWS], f32, isOutput=False)
    qc = nc.declare_dram_parameter("qc", [_P, 2], f32, isOutput=False)
    poff = nc.declare_dram_parameter("poff", [_P, 1], f32, isOutput=False)
    rsel = nc.declare_dram_parameter("rsel", [_P, _ROWS], f32, isOutput=False)
    out = nc.declare_dram_parameter("out", [1, 1], f32, isOutput=True)

    with tile.TileContext(nc) as tc:
        with (
            tc.tile_pool(name="const", bufs=1) as cpool,
            tc.tile_pool(name="a", bufs=8) as apool,
            tc.tile_pool(name="e", bufs=2) as epool,
            tc.tile_pool(name="scr", bufs=2) as spool,
            tc.tile_pool(name="tchunk", bufs=8) as tpool,
            tc.tile_pool(name="st", bufs=1) as stpool,
            tc.tile_pool(name="ps", bufs=1, space="PSUM") as ppool,
        ):
            # constants: xf issued just after the first chunk pair; others
            # (needed only in the tail) go on the slow gpsimd path
            xf_t = cpool.tile([_P, _CHUNK], f32)
            yw_t = cpool.tile([_P, _HROWS], f32)
            nc.gpsimd.dma_start(yw_t[:], yw[:])
            qc_t = cpool.tile([_P, 2], f32)
            nc.gpsimd.dma_start(qc_t[:], qc[:])
            poff_t = cpool.tile([_P, 1], f32)
            nc.gpsimd.dma_start(poff_t[:], poff[:])
            rsel_t = cpool.tile([_P, _ROWS], f32)
            nc.gpsimd.dma_start(rsel_t[:], rsel[:])

            ncnk = len(_SIZES)
            offs = [0]
            for sz in _SIZES:
                offs.append(offs[-1] + sz)
            scol = stpool.tile([_P, _HROWS], f32)   # per-heatmap-row exp sums
            sxcol = stpool.tile([_P, ncnk], f32)
            rall = stpool.tile([_P, _HROWS], f32)   # per-heatmap-row target max

            # Mildly T-favored interleave: the target stream finishes at
            # ~80% of the total volume so its argmax tail (row-max merge +
            # ~5us indirect gather + coords) hides under the tail of the
            # input stream, which finishes last with its smallest chunks.
            sched = [("t", 0), ("a", 0), ("t", 1), ("a", 1), ("t", 2),
                     ("a", 2), ("t", 3), ("a", 3), ("t", 4), ("t", 5),
                     ("a", 4), ("t", 6), ("t", 7), ("a", 5), ("t", 8),
                     ("t", 9), ("t", 10), ("a", 6), ("a", 7), ("a", 8),
                     ("a", 9), ("a", 10)]
            assert len(sched) == 2 * ncnk

            t_tiles, a_tiles = {}, {}
            for kind, c in sched:
                o, sz = offs[c], _SIZES[c]
                if kind == "t":
                    t_t = tpool.tile([_P, sz], f32, tag="tch")
                    nc.sync.dma_start(t_t[:], tgt[:, o:o + sz])
                    t_tiles[c] = t_t
                else:
                    a_t = apool.tile([_P, sz], f32, tag="ach")
                    nc.sync.dma_start(a_t[:], inp[:, o:o + sz])
                    a_tiles[c] = a_t
                    if c == 0:
                        nc.sync.dma_start(xf_t[:], xf[:])

            for kind, c in sched:
                o, sz = offs[c], _SIZES[c]
                nrow = sz // _W
                r0 = o // _W
                if kind == "t":
                    nc.vector.tensor_reduce(
                        rall[:, r0:r0 + nrow],
                        t_tiles[c][:].rearrange("p (k j) -> p k j", j=_W),
                        axis=Ax.X, op=Alu.max)
                    continue
                a_t = a_tiles[c]
                e_t = epool.tile([_P, sz], f32, tag="ech")
                for j in range(nrow):
                    r = r0 + j
                    nc.scalar.activation(
                        e_t[:, j * _W:(j + 1) * _W],
                        a_t[:, j * _W:(j + 1) * _W],
                        Act.Exp,
                        accum_out=scol[:, r:r + 1],
                    )
                ex_t = spool.tile([_P, sz], f32, tag="exch")
                nc.vector.scalar_tensor_tensor(
                    ex_t[:], e_t[:], 1.0, xf_t[:, 0:sz],
                    op0=Alu.mult, op1=Alu.mult,
                    accum_out=sxcol[:, c:c + 1],
                )

            # ---- per-partition argmax: V, row k*, column j* ----
            m8p = stpool.tile([_P, 8], f32)
            nc.vector.memset(m8p[:], -1.0e30)
            nc.vector.tensor_reduce(m8p[:, 0:1], rall[:], axis=Ax.X, op=Alu.max)  # V
            k8 = stpool.tile([_P, 8], u32)
            nc.vector.max_index(k8[:], m8p[:], rall[:])
            kf = stpool.tile([_P, 2], f32)
            nc.vector.tensor_copy(kf[:, 0:1], k8[:, 0:1])
            ridxf = stpool.tile([_P, 1], f32)
            nc.vector.tensor_scalar(ridxf[:], kf[:, 0:1], poff_t[:, 0:1], None, Alu.add)
            ridx = stpool.tile([_P, 1], u32)
            nc.vector.tensor_copy(ridx[:], ridxf[:])
            grow = stpool.tile([_P, _W], f32)
            nc.gpsimd.indirect_dma_start(
                out=grow[:], out_offset=None,
                in_=tgt[:].rearrange("p (k j) -> (p k) j", j=_W),
                in_offset=bass.IndirectOffsetOnAxis(ap=ridx[:], axis=0),
            )
            j8 = stpool.tile([_P, 8], u32)
            nc.vector.max_index(j8[:], m8p[:], grow[:])

            # coords (f32 exact integers)
            nc.vector.tensor_copy(kf[:, 1:2], j8[:, 0:1])
            # stats tile X: cols [S, Sx, Sy, TXw, TYw, G, TX, TY]
            X = stpool.tile([_P, 8], f32)
            nc.vector.reduce_sum(X[:, 0:1], scol[:], axis=Ax.X)           # S
            nc.vector.reduce_sum(X[:, 1:2], sxcol[:], axis=Ax.X)          # Sx
            sy_scr = stpool.tile([_P, _HROWS], f32)
            nc.vector.scalar_tensor_tensor(
                sy_scr[:], scol[:], 1.0, yw_t[:],
                op0=Alu.mult, op1=Alu.mult, accum_out=X[:, 2:3])          # Sy
            # G = q*16384 + 256*k + j
            gtmp = stpool.tile([_P, 1], f32)
            nc.vector.tensor_scalar(gtmp[:], kf[:, 0:1], 256.0, qc_t[:, 0:1],
                                    Alu.mult, Alu.add)
            nc.vector.tensor_add(X[:, 5:6], gtmp[:], kf[:, 1:2])          # G
            # TX = (j+1)/256 ; TY = (k + q*64 + 1)/256
            nc.vector.tensor_scalar(X[:, 6:7], kf[:, 1:2], 1.0, 1.0 / _W,
                                    Alu.add, Alu.mult)                    # TX
            nc.vector.tensor_scalar(X[:, 7:8], kf[:, 0:1], qc_t[:, 1:2], 1.0 / _H,
                                    Alu.add, Alu.mult)                    # TY

            # ---- cross-quarter merge via stream_shuffle in groups of 4 ----
            sh = stpool.tile([_P, 6], f32)
            nc.vector.stream_shuffle(sh[:, 0:1], m8p[:, 0:1], _ROT1)
            nc.vector.tensor_max(sh[:, 1:2], m8p[:, 0:1], sh[:, 0:1])
            nc.vector.stream_shuffle(sh[:, 2:3], sh[:, 1:2], _ROT2)
            nc.vector.tensor_max(sh[:, 2:3], sh[:, 1:2], sh[:, 2:3])      # Vrow
            nm = stpool.tile([_P, 1], f32)
            nc.vector.tensor_tensor(nm[:], m8p[:, 0:1], sh[:, 2:3], op=Alu.is_lt)
            ge = stpool.tile([_P, 4], f32)
            nc.vector.scalar_tensor_tensor(
                ge[:, 0:1], nm[:], 1.0e9, X[:, 5:6], op0=Alu.mult, op1=Alu.add)
            nc.vector.stream_shuffle(ge[:, 1:2], ge[:, 0:1], _ROT1)
            nc.vector.tensor_tensor(ge[:, 1:2], ge[:, 0:1], ge[:, 1:2], op=Alu.min)
            nc.vector.stream_shuffle(ge[:, 2:3], ge[:, 1:2], _ROT2)
            nc.vector.tensor_tensor(ge[:, 2:3], ge[:, 1:2], ge[:, 2:3], op=Alu.min)  # Grow
            wsel = stpool.tile([_P, 1], f32)
            nc.vector.tensor_tensor(wsel[:], X[:, 5:6], ge[:, 2:3], op=Alu.is_equal)
            nc.vector.tensor_mul(X[:, 3:4], X[:, 6:7], wsel[:])          # TXw
            nc.vector.tensor_mul(X[:, 4:5], X[:, 7:8], wsel[:])          # TYw

            # ---- fold partitions -> heatmaps with PE, finish per-row math ----
            ps1 = ppool.tile([_ROWS, 5], f32)
            nc.tensor.matmul(ps1[:], rsel_t[:], X[:, 0:5], start=True, stop=True)
            fin = stpool.tile([_ROWS, 16], f32)
            nc.vector.tensor_copy(fin[:, 0:5], ps1[:])
            nc.vector.reciprocal(fin[:, 5:6], fin[:, 0:1])                # 1/S
            nc.vector.tensor_mul(fin[:, 6:7], fin[:, 1:2], fin[:, 5:6])   # predx
            nc.vector.tensor_mul(fin[:, 7:8], fin[:, 2:3], fin[:, 5:6])   # predy
            nc.vector.tensor_sub(fin[:, 8:9], fin[:, 3:4], fin[:, 6:7])   # dx
            nc.vector.tensor_sub(fin[:, 9:10], fin[:, 4:5], fin[:, 7:8])  # dy
            dsq = stpool.tile([_ROWS, 2], f32)
            nc.vector.scalar_tensor_tensor(
                dsq[:], fin[:, 8:10], 1.0, fin[:, 8:10],
                op0=Alu.mult, op1=Alu.mult, accum_out=fin[:, 10:11])      # dx^2+dy^2
            nc.scalar.sqrt(fin[:, 11:12], fin[:, 10:11])                  # ed
            ones32 = stpool.tile([_ROWS, 1], f32)
            nc.vector.memset(ones32[:], 1.0)
            ps2 = ppool.tile([1, 1], f32)
            nc.tensor.matmul(ps2[:], ones32[:], fin[:, 11:12], start=True, stop=True)
            pr = stpool.tile([1, 1], f32)
            nc.vector.tensor_copy(pr[:], ps2[:])
            nc.sync.dma_start(out[:], pr[:])

    return nc


def _constants():
    p = np.arange(_P)
    q = (p % 4).astype(np.float32)
    xrow = ((np.arange(_CHUNK) % _W) + 1).astype(np.float32) / _W
    xf = np.tile(xrow[None, :], (_P, 1)).astype(np.float32)
    k = np.arange(_HROWS, dtype=np.float32)
    yw = ((q[:, None] * 64.0 + k[None, :] + 1.0) / _H).astype(np.float32)
    qc = np.stack([q * 16384.0, q * 64.0 + 1.0], axis=1).astype(np.float32)
    poff = (p.astype(np.float32) * _HROWS).reshape(_P, 1)
    rsel = (p[:, None] // 4 == np.arange(_ROWS)[None, :]).astype(np.float32)
    return xf, yw, qc, np.ascontiguousarray(poff), np.ascontiguousarray(rsel)


def _ensure_ntff_hook():
    """Provide antenv.axon_hooks with a ctypes NTFF profile hook if missing."""
    import sys
    import types
    import ctypes
    import contextlib

    try:
        from antenv.axon_hooks import get_axon_ntff_profile_hook  # noqa: F401
        return
    except ImportError:
        pass

    so_path = "/opt/axon/libaxon_pjrt.so"
    try:
        lib = ctypes.CDLL(so_path)
    except OSError:
        return
    if not hasattr(lib, "axon_start_nrt_profile"):
        return
    lib.axon_start_nrt_profile.argtypes = [
        ctypes.POINTER(ctypes.c_int64), ctypes.c_size_t]
    lib.axon_start_nrt_profile.restype = ctypes.c_int64
    lib.axon_stop_nrt_profile.argtypes = [ctypes.c_char_p]
    lib.axon_stop_nrt_profile.restype = ctypes.c_int64

    @contextlib.contextmanager
    def _hook(output_dir, device_ids):
        import jax
        jax.devices()
        if device_ids:
            ids = (ctypes.c_int64 * len(device_ids))(*device_ids)
            rc = lib.axon_start_nrt_profile(ids, len(device_ids))
        else:
            rc = lib.axon_start_nrt_profile(None, 0)
        if rc != 0:
            raise RuntimeError(f"axon_start_nrt_profile rc={rc}")
        try:
            yield
        finally:
            n = lib.axon_stop_nrt_profile(str(output_dir).encode())
            if n < 0:
                raise RuntimeError(f"axon_stop_nrt_profile rc={n}")

    mod = types.ModuleType("antenv.axon_hooks")
    mod.get_axon_ntff_profile_hook = lambda: _hook
    mod.set_axon_ntff_profile_hook = lambda h: None
    sys.modules["antenv.axon_hooks"] = mod


def _run(inputs, trace=False):
    from concourse import bass_utils

    if trace:
        _ensure_ntff_hook()

    global _compiled
    if _compiled is None:
        _compiled = _build_program()
    nc = _compiled

    inp = np.ascontiguousarray(inputs["input"], dtype=np.float32)
    tgt = np.ascontiguousarray(inputs["target"], dtype=np.float32)
    xf, yw, qc, poff, rsel = _constants()

    per_core_b = _B // _NCORES
    in_maps = []
    for core in range(_NCORES):
        sl = slice(core * per_core_b, (core + 1) * per_core_b)
        in_maps.append({
            "inp": np.ascontiguousarray(inp[sl]).reshape(_P, _FREE),
            "tgt": np.ascontiguousarray(tgt[sl]).reshape(_P, _FREE),
            "xf": xf, "yw": yw, "qc": qc, "poff": poff, "rsel": rsel,
        })

    res = bass_utils.run_bass_kernel_spmd(
        nc, in_maps, core_ids=list(range(_NCORES)), trace=trace)
    total = np.float64(0.0)
    for r in res.results:
        total += np.float64(r["out"].reshape(-1)[0])
    loss = np.array([total / _B], dtype=np.float32)
    return loss, res


def kernel(**inputs):
    loss, _ = _run(inputs)
    return loss


# revision 25
# speedup vs baseline: 1.1409x; 1.1409x over previous
"""DSNT double loss kernel for Trainium2 (Bass/Tile), data-parallel over batch.

Problem: input/target [B=32, C=8, H=256, W=256] f32.
  probs = softmax(input.reshape(B,C,H*W)); pred_x = sum(probs * (w+1)/W);
  pred_y = sum(probs * (h+1)/H); am = argmax(target); true coords from am;
  loss = sum(sqrt((tx-px)^2+(ty-py)^2)) / B, shape (1,).

Sharding: batch dim across 8 cores (4 batches/core = 32 heatmaps/core).
Per-core layout: the 32 heatmaps (65536 px each) are viewed as a [128, 16384]
f32 block: partition p holds quarter q = p%4 of heatmap r = p//4 (contiguous
reshape).

Per-core dataflow (one pass over the data, DMA-bound):
  - input chunks: ACT computes exp with per-heatmap-row accumulators (scol);
    DVE does one fused multiply+accumulate pass against the x-weights (Sx).
  - target chunks: DVE computes per-heatmap-row maxima (Rall) in one
    tensor_reduce per chunk. The argmax is then recovered cheaply: V =
    max(Rall); k* = find_index8(V, Rall) gives the row; one indirect DMA
    gathers that 256-px row from DRAM; j* = find_index8(V, row) gives the
    column. Exact because the target has no duplicated per-line maxima.
  - cross-quarter (4 partitions per heatmap) merge via stream_shuffle
    rotations inside partition groups of 4, then one PE matmul against a
    0/1 row-selector folds [128] partitions to [32] heatmaps; a second
    1-column matmul sums the 32 EDs to the scalar partial.
"""

import numpy as np

_B, _C, _H, _W = 32, 8, 256, 256
_NCORES = 8
_P = 128
_ROWS = (_B // _NCORES) * _C          # 32 heatmaps per core
_ROWLEN = _H * _W                     # 65536
_FREE = _ROWS * _ROWLEN // _P         # 16384 elements per partition
_CHUNK = 2048
_NCHUNK = _FREE // _CHUNK             # 8
_HROWS = _FREE // _W                  # 64 heatmap-rows per partition
# last input chunk is processed in 512-wide sub-chunks for a short drain
_SUB = 512
_NSUB = _CHUNK // _SUB

_compiled = None
_tile_patched = False


def _patch_tile_single_wait():
    """The walrus build in this image encodes at most ONE semaphore wait per
    instruction ("Too many sync wait commands" / "ISA wrong length" errors in
    codegen otherwise). Split any multi-wait instruction into single-wait
    same-engine NOPs inserted immediately before it, and do the same for the
    kernel-tail drain."""
    global _tile_patched
    if _tile_patched:
        return
    _tile_patched = True

    import bass_rust
    from concourse import tile, mybir
    from concourse.vector_clock import ScopedClock

    ctr = [0]

    def split_waits(tc, inst):
        si = inst.sync_info
        if si is None or not si.on_wait or len(si.on_wait) <= 1:
            return
        wl = list(si.on_wait)
        si.on_wait = wl[-1:]
        for w in wl[:-1]:
            ctr[0] += 1
            nop = mybir.InstNoOp(name=f"wsplit-{ctr[0]}", ins=[], outs=[])
            nop.engine = inst.engine
            nop.sync_info = bass_rust.SyncInfo(on_wait=[w], on_update=[])
            tc._add_instruction(nop)

    orig_commit = tile.TileContext._commit_instruction

    def commit(self, inst, lazy_reg_writes=True):
        si = inst.sync_info
        if (si is not None and si.on_wait and len(si.on_wait) > 1
                and inst.engine != mybir.EngineType.Unassigned):
            split_waits(self, inst)
        return orig_commit(self, inst, lazy_reg_writes)

    tile.TileContext._commit_instruction = commit

    def drain_and_barrier(self, tick_clock, wait_clock):
        nc = self.nc
        drain_inst = nc.sync.drain()
        wait_clock.add_sem_waits(
            drain_inst.ins, ScopedClock({None: tick_clock.global_clock}))
        si = drain_inst.ins.sync_info
        wl = list(si.on_wait or []) if si else []
        if len(wl) > 1:
            si.on_wait = wl[:1]
            for w in wl[1:]:
                ctr[0] += 1
                nop = mybir.InstNoOp(name=f"dsplit-{ctr[0]}", ins=[], outs=[])
                nop.engine = mybir.EngineType.SP
                nop.sync_info = bass_rust.SyncInfo(on_wait=[w], on_update=[])
                self._add_instruction(nop)
        nc.all_engine_barrier()
        assert self.sems is not None
        popped = nc._tile_sem_poison_stack.pop()
        assert popped is self._sem_poison
        nc.clear_and_free_semaphores(list(self.sems.allocated().values()))
        nc.all_engine_barrier()

    tile.TileContext._drain_and_barrier = drain_and_barrier


# stream_shuffle masks: rotate by 1 / 2 within each partition group of 4
_ROT1 = [(i & ~3) | ((i + 1) & 3) for i in range(32)]
_ROT2 = [(i & ~3) | ((i + 2) & 3) for i in range(32)]


def _build_program():
    from concourse import bass, tile, mybir

    _patch_tile_single_wait()

    f32 = mybir.dt.float32
    u32 = mybir.dt.uint32
    Alu = mybir.AluOpType
    Act = mybir.ActivationFunctionType
    Ax = mybir.AxisListType

    nc = bass.Bass()
    inp = nc.declare_dram_parameter("inp", [_P, _FREE], f32, isOutput=False)
    tgt = nc.declare_dram_parameter("tgt", [_P, _FREE], f32, isOutput=False)
    xf = nc.declare_dram_parameter("xf", [_P, _CHUNK], f32, isOutput=False)
    yw = nc.declare_dram_parameter("yw", [_P, _HROWS], f32, isOutput=False)
    qc = nc.declare_dram_parameter("qc", [_P, 2], f32, isOutput=False)
    poff = nc.declare_dram_parameter("poff", [_P, 1], f32, isOutput=False)
    rsel = nc.declare_dram_parameter("rsel", [_P, _ROWS], f32, isOutput=False)
    out = nc.declare_dram_parameter("out", [1, 1], f32, isOutput=True)

    with tile.TileContext(nc) as tc:
        with (
            tc.tile_pool(name="const", bufs=1) as cpool,
            tc.tile_pool(name="a", bufs=8) as apool,
            tc.tile_pool(name="e", bufs=2) as epool,
            tc.tile_pool(name="scr", bufs=2) as spool,
            tc.tile_pool(name="tchunk", bufs=8) as tpool,
            tc.tile_pool(name="st", bufs=1) as stpool,
            tc.tile_pool(name="ps", bufs=1, space="PSUM") as ppool,
        ):
            # constants: xf issued just after the first chunk pair; others
            # (needed only in the tail) go on the slow gpsimd path
            xf_t = cpool.tile([_P, _CHUNK], f32)
            yw_t = cpool.tile([_P, _HROWS], f32)
            nc.gpsimd.dma_start(yw_t[:], yw[:])
            qc_t = cpool.tile([_P, 2], f32)
            nc.gpsimd.dma_start(qc_t[:], qc[:])
            poff_t = cpool.tile([_P, 1], f32)
            nc.gpsimd.dma_start(poff_t[:], poff[:])
            rsel_t = cpool.tile([_P, _ROWS], f32)
            nc.gpsimd.dma_start(rsel_t[:], rsel[:])

            scol = stpool.tile([_P, _HROWS], f32)   # per-heatmap-row exp sums
            sxcol = stpool.tile([_P, _NCHUNK - 1 + _NSUB], f32)
            rall = stpool.tile([_P, _HROWS], f32)   # per-heatmap-row target max
            rows_per_chunk = _CHUNK // _W  # 8

            # issue every stream DMA up front (bufs=8: no slot waits); the
            # SP HWDGE ring paces itself off completions. The last input
            # chunk is split 4x512 so its compute tail is short.
            t_tiles, a_tiles, a_subs = [], [], []
            for c in range(_NCHUNK):
                t_t = tpool.tile([_P, _CHUNK], f32, tag="tch")
                nc.sync.dma_start(t_t[:], tgt[:, c * _CHUNK:(c + 1) * _CHUNK])
                t_tiles.append(t_t)
                if c < _NCHUNK - 1:
                    a_t = apool.tile([_P, _CHUNK], f32, tag="ach")
                    nc.sync.dma_start(a_t[:], inp[:, c * _CHUNK:(c + 1) * _CHUNK])
                    a_tiles.append(a_t)
                else:
                    for sv in range(_NSUB):
                        o = c * _CHUNK + sv * _SUB
                        a_s = apool.tile([_P, _SUB], f32, tag="ach")
                        nc.sync.dma_start(a_s[:], inp[:, o:o + _SUB])
                        a_subs.append(a_s)
                if c == 0:
                    nc.sync.dma_start(xf_t[:], xf[:])

            # pre-warm the sqrt activation table during the ramp
            warm = stpool.tile([1, 1], f32)
            nc.vector.memset(warm[:], 1.0)
            nc.scalar.sqrt(warm[:], warm[:])

            def do_exp_stt(a_ap, o, sz, sx_col):
                nrow = sz // _W
                r0 = o // _W
                e_t = epool.tile([_P, sz], f32, tag="ech")
                for j in range(nrow):
                    nc.scalar.activation(
                        e_t[:, j * _W:(j + 1) * _W],
                        a_ap[:, j * _W:(j + 1) * _W],
                        Act.Exp,
                        accum_out=scol[:, r0 + j:r0 + j + 1],
                    )
                ex_t = spool.tile([_P, sz], f32, tag="exch")
                nc.vector.scalar_tensor_tensor(
                    ex_t[:], e_t[:], 1.0, xf_t[:, 0:sz],
                    op0=Alu.mult, op1=Alu.mult,
                    accum_out=sxcol[:, sx_col:sx_col + 1],
                )

            for c in range(_NCHUNK):
                nc.vector.tensor_reduce(
                    rall[:, c * rows_per_chunk:(c + 1) * rows_per_chunk],
                    t_tiles[c][:].rearrange("p (k j) -> p k j", j=_W),
                    axis=Ax.X, op=Alu.max)
                if c < _NCHUNK - 1:
                    do_exp_stt(a_tiles[c], c * _CHUNK, _CHUNK, c)

            # argmax chain part 1 right after the last T-reduce: the DVE is
            # in-order, so this must precede the remaining input-side work
            # for the gather to fire early on gpsimd
            m8p = stpool.tile([_P, 8], f32)
            nc.vector.memset(m8p[:], -1.0e30)
            nc.vector.tensor_reduce(m8p[:, 0:1], rall[:], axis=Ax.X, op=Alu.max)  # V
            k8 = stpool.tile([_P, 8], u32)
            nc.vector.max_index(k8[:], m8p[:], rall[:])
            kf = stpool.tile([_P, 2], f32)
            nc.vector.tensor_copy(kf[:, 0:1], k8[:, 0:1])
            ridxf = stpool.tile([_P, 1], f32)
            nc.vector.tensor_scalar(ridxf[:], kf[:, 0:1], poff_t[:, 0:1], None, Alu.add)
            ridx = stpool.tile([_P, 1], u32)
            nc.vector.tensor_copy(ridx[:], ridxf[:])
            grow = stpool.tile([_P, _W], f32)
            nc.gpsimd.indirect_dma_start(
                out=grow[:], out_offset=None,
                in_=tgt[:].rearrange("p (k j) -> (p k) j", j=_W),
                in_offset=bass.IndirectOffsetOnAxis(ap=ridx[:], axis=0),
            )

            # last input chunk in short sub-chunks
            cbase = _NCHUNK - 1
            for sv in range(_NSUB):
                o = cbase * _CHUNK + sv * _SUB
                do_exp_stt(a_subs[sv], o, _SUB, cbase + sv)

            # ---- argmax part 2: column j* from the gathered row ----
            j8 = stpool.tile([_P, 8], u32)
            nc.vector.max_index(j8[:], m8p[:], grow[:])

            # coords (f32 exact integers)
            nc.vector.tensor_copy(kf[:, 1:2], j8[:, 0:1])
            # stats tile X: cols [S, Sx, Sy, TXw, TYw, G, TX, TY]
            X = stpool.tile([_P, 8], f32)
            nc.vector.reduce_sum(X[:, 0:1], scol[:], axis=Ax.X)           # S
            nc.vector.reduce_sum(X[:, 1:2], sxcol[:], axis=Ax.X)          # Sx
            sy_scr = stpool.tile([_P, _HROWS], f32)
            nc.vector.scalar_tensor_tensor(
                sy_scr[:], scol[:], 1.0, yw_t[:],
                op0=Alu.mult, op1=Alu.mult, accum_out=X[:, 2:3])          # Sy
            # G = q*16384 + 256*k + j
            gtmp = stpool.tile([_P, 1], f32)
            nc.vector.tensor_scalar(gtmp[:], kf[:, 0:1], 256.0, qc_t[:, 0:1],
                                    Alu.mult, Alu.add)
            nc.vector.tensor_add(X[:, 5:6], gtmp[:], kf[:, 1:2])          # G
            # TX = (j+1)/256 ; TY = (k + q*64 + 1)/256
            nc.vector.tensor_scalar(X[:, 6:7], kf[:, 1:2], 1.0, 1.0 / _W,
                                    Alu.add, Alu.mult)                    # TX
            nc.vector.tensor_scalar(X[:, 7:8], kf[:, 0:1], qc_t[:, 1:2], 1.0 / _H,
                                    Alu.add, Alu.mult)                    # TY

            # ---- cross-quarter merge via stream_shuffle in groups of 4 ----
            sh = stpool.tile([_P, 6], f32)
            nc.vector.stream_shuffle(sh[:, 0:1], m8p[:, 0:1], _ROT1)
            nc.vector.tensor_max(sh[:, 1:2], m8p[:, 0:1], sh[:, 0:1])
            nc.vector.stream_shuffle(sh[:, 2:3], sh[:, 1:2], _ROT2)
            nc.vector.tensor_max(sh[:, 2:3], sh[:, 1:2], sh[:, 2:3])      # Vrow
            nm = stpool.tile([_P, 1], f32)
            nc.vector.tensor_tensor(nm[:], m8p[:, 0:1], sh[:, 2:3], op=Alu.is_lt)
            ge = stpool.tile([_P, 4], f32)
            nc.vector.scalar_tensor_tensor(
                ge[:, 0:1], nm[:], 1.0e9, X[:, 5:6], op0=Alu.mult, op1=Alu.add)
            nc.vector.stream_shuffle(ge[:, 1:2], ge[:, 0:1], _ROT1)
            nc.vector.tensor_tensor(ge[:, 1:2], ge[:, 0:1], ge[:, 1:2], op=Alu.min)
            nc.vector.stream_shuffle(ge[:, 2:3], ge[:, 1:2], _ROT2)
            nc.vector.tensor_tensor(ge[:, 2:3], ge[:, 1:2], ge[:, 2:3], op=Alu.min)  # Grow
            wsel = stpool.tile([_P, 1], f32)
            nc.vector.tensor_tensor(wsel[:], X[:, 5:6], ge[:, 2:3], op=Alu.is_equal)
            nc.vector.tensor_mul(X[:, 3:4], X[:, 6:7], wsel[:])          # TXw
            nc.vector.tensor_mul(X[:, 4:5], X[:, 7:8], wsel[:])          # TYw

            # ---- fold partitions -> heatmaps with PE, finish per-row math ----
            ps1 = ppool.tile([_ROWS, 5], f32)
            nc.tensor.matmul(ps1[:], rsel_t[:], X[:, 0:5], start=True, stop=True)
            fin = stpool.tile([_ROWS, 16], f32)
            nc.vector.tensor_copy(fin[:, 0:5], ps1[:])
            nc.vector.reciprocal(fin[:, 5:6], fin[:, 0:1])                # 1/S
            nc.vector.tensor_mul(fin[:, 6:7], fin[:, 1:2], fin[:, 5:6])   # predx
            nc.vector.tensor_mul(fin[:, 7:8], fin[:, 2:3], fin[:, 5:6])   # predy
            nc.vector.tensor_sub(fin[:, 8:9], fin[:, 3:4], fin[:, 6:7])   # dx
            nc.vector.tensor_sub(fin[:, 9:10], fin[:, 4:5], fin[:, 7:8])  # dy
            dsq = stpool.tile([_ROWS, 2], f32)
            nc.vector.scalar_tensor_tensor(
                dsq[:], fin[:, 8:10], 1.0, fin[:, 8:10],
                op0=Alu.mult, op1=Alu.mult, accum_out=fin[:, 10:11])      # dx^2+dy^2
            nc.scalar.sqrt(fin[:, 11:12], fin[:, 10:11])                  # ed
            ones32 = stpool.tile([_ROWS, 1], f32)
            nc.vector.memset(ones32[:], 1.0)
            ps2 = ppool.tile([1, 1], f32)
            nc.tensor.matmul(ps2[:], ones32[:], fin[:, 11:12], start=True, stop=True)
            pr = stpool.tile([1, 1], f32)
            nc.vector.tensor_copy(pr[:], ps2[:])
            nc.sync.dma_start(out[:], pr[:])

    return nc


def _constants():
    p = np.arange(_P)
    q = (p % 4).astype(np.float32)
    xrow = ((np.arange(_CHUNK) % _W) + 1).astype(np.float32) / _W
    xf = np.tile(xrow[None, :], (_P, 1)).astype(np.float32)
    k = np.arange(_HROWS, dtype=np.float32)
    yw = ((q[:, None] * 64.0 + k[None, :] + 1.0) / _H).astype(np.float32)
    qc = np.stack([q * 16384.0, q * 64.0 + 1.0], axis=1).astype(np.float32)
    poff = (p.astype(np.float32) * _HROWS).reshape(_P, 1)
    rsel = (p[:, None] // 4 == np.arange(_ROWS)[None, :]).astype(np.float32)
    return xf, yw, qc, np.ascontiguousarray(poff), np.ascontiguousarray(rsel)


def _ensure_ntff_hook():
    """Provide antenv.axon_hooks with a ctypes NTFF profile hook if missing."""
    import sys
    import types
    import ctypes
    import contextlib

    try:
        from antenv.axon_hooks import get_axon_ntff_profile_hook  # noqa: F401
        return
    except ImportError:
        pass

    so_path = "/opt/axon/libaxon_pjrt.so"
    try:
        lib = ctypes.CDLL(so_path)
    except OSError:
        return
    if not hasattr(lib, "axon_start_nrt_profile"):
        return
    lib.axon_start_nrt_profile.argtypes = [
        ctypes.POINTER(ctypes.c_int64), ctypes.c_size_t]
    lib.axon_start_nrt_profile.restype = ctypes.c_int64
    lib.axon_stop_nrt_profile.argtypes = [ctypes.c_char_p]
    lib.axon_stop_nrt_profile.restype = ctypes.c_int64

    @contextlib.contextmanager
    def _hook(output_dir, device_ids):
        import jax
        jax.devices()
        if device_ids:
            ids = (ctypes.c_int64 * len(device_ids))(*device_ids)
            rc = lib.axon_start_nrt_profile(ids, len(device_ids))
        else:
            rc = lib.axon_start_nrt_profile(None, 0)
        if rc != 0:
            raise RuntimeError(f"axon_start_nrt_profile rc={rc}")
        try:
            yield
        finally:
            n = lib.axon_stop_nrt_profile(str(output_dir).encode())
            if n < 0:
                raise RuntimeError(f"axon_stop_nrt_profile rc={n}")

    mod = types.ModuleType("antenv.axon_hooks")
    mod.get_axon_ntff_profile_hook = lambda: _hook
    mod.set_axon_ntff_profile_hook = lambda h: None
    sys.modules["antenv.axon_hooks"] = mod


def _run(inputs, trace=False):
    from concourse import bass_utils

    if trace:
        _ensure_ntff_hook()

    global _compiled
    if _compiled is None:
        _compiled = _build_program()
    nc = _compiled

    inp = np.ascontiguousarray(inputs["input"], dtype=np.float32)
    tgt = np.ascontiguousarray(inputs["target"], dtype=np.float32)
    xf, yw, qc, poff, rsel = _constants()

    per_core_b = _B // _NCORES
    in_maps = []
    for core in range(_NCORES):
        sl = slice(core * per_core_b, (core + 1) * per_core_b)
        in_maps.append({
            "inp": np.ascontiguousarray(inp[sl]).reshape(_P, _FREE),
            "tgt": np.ascontiguousarray(tgt[sl]).reshape(_P, _FREE),
            "xf": xf, "yw": yw, "qc": qc, "poff": poff, "rsel": rsel,
        })

    res = bass_utils.run_bass_kernel_spmd(
        nc, in_maps, core_ids=list(range(_NCORES)), trace=trace)
    total = np.float64(0.0)
    for r in res.results:
        total += np.float64(r["out"].reshape(-1)[0])
    loss = np.array([total / _B], dtype=np.float32)
    return loss, res


def kernel(**inputs):
    loss, _ = _run(inputs)
    return loss
